# revision 37
# baseline (speedup 1.0000x reference)
"""Trainium2 Bass kernel for Batch_MGCN_Value (multi-type GCN value network).

Data-parallel over batch: 16 batches -> 8 NeuronCores (2 per core).
Activations are feature-major with both local batches folded onto partitions
([128 = feat + 64*bl, 1024 = node]).  MLP layers use block-diagonal weights
(lhsT [128, 128] = diag(W, W)) so one f32r matmul serves both batches and the
PSUM write stays at column position 0 (an f32r ISA requirement).  Message
passing contracts against host-transposed adjacency in fp16 (exact for 0/1).
The per-channel take_along_axis gather runs on GPSIMD ap_gather against a
partition-replicated copy of gcn_out produced by a replicated-weight matmul.
"""
import numpy as np
from contextlib import ExitStack

import concourse.bass as bass
import concourse.mybir as mybir
import concourse.tile as tile
from concourse import bacc
from concourse.bass_utils import run_bass_kernel_spmd

FP32 = mybir.dt.float32
F32R = mybir.dt.float32r
FP16 = mybir.dt.float16
I16 = mybir.dt.int16
LR = mybir.ActivationFunctionType.Lrelu
IDENT = mybir.ActivationFunctionType.Identity

NCORES = 8
B, T, N, F, H, HID, OUT, S = 16, 2, 1024, 16, 64, 64, 32, 256
BPC = B // NCORES          # batches per core
NSTEPS = 2
ALPHA = 0.01
KC = N // 128              # 8 k-chunks of the node dim
NH = 512                   # matmul free-dim half
USE_DVE_LRELU = False      # DVE 2-op lrelu offload measured slower in the model

# ------------------------------------------------------------- weight packing
# (name, rows, cols, dup): dup=True entries are replicated to rows 64:128 so a
# bl=1 / t=1 matmul can read the weight at base partition 64.

WR_SPEC = []
for t in range(T):
    WR_SPEC += [(f"in{t}_bd0", 2 * F, 128, False),
                (f"in{t}_bd1", 128, 128, False),
                (f"in{t}_bd2", 128, 128, False)]
WR_SPEC += [("msg1cat", H, 128, True), ("msg2bd", 128, 128, False)]
for t in range(T):
    WR_SPEC += [(f"msg3_{t}", HID, H, True)]
WR_SPEC += [("agg_bd0", 128, 128, False), ("agg_bd1", 128, 128, False),
            ("agg_bd2", 128, 128, False)]
WR_SPEC += [("out_bd0", 128, 128, False), ("out_bd1", 128, 128, False)]
for og in range(4):
    WR_SPEC += [(f"out3rep_{og}", HID, 128, True)]
WR_SPEC += [("fL2bd", 128, 128, False), ("fL3bd", 128, 2 * OUT, False)]

W32_SPEC = [(f"f1rep_{og}", 128, HID) for og in range(4)]
W32_SPEC += [("g_0", OUT, HID), ("g_1", HID, HID), ("g_2", HID, 1)]

B_SPEC = []
for t in range(T):
    B_SPEC += [(f"bin{t}_0", 128), (f"bin{t}_1", 128), (f"bin{t}_2", 128)]
B_SPEC += [("bmsg1cat", 128), ("bmsg2cat", 128), ("bmsg3cat", 128)]
B_SPEC += [("bagg_0", 128), ("bagg_1", 128), ("bagg_2", 128)]
B_SPEC += [("bout_0", 128), ("bout_1", 128)]
for og in range(4):
    B_SPEC += [(f"bout3rep_{og}", 128)]
B_SPEC += [("bf_0", 64), ("bf_1", 128), ("bf_2", 64)]
B_SPEC += [("bg_0", HID), ("bg_1", HID), ("bg_2", 1)]


def _layout(spec):
    ofs, c = {}, 0
    for e in spec:
        ofs[e[0]] = c
        c += e[2]
    return ofs, c


WR_OFS, WR_COLS = _layout(WR_SPEC)
W32_OFS, W32_COLS = _layout(W32_SPEC)
# hot region = in-MLP weights (first 6 entries); everything else cold
HOT_COLS = _layout(WR_SPEC[:6])[1]
# cold pack layout: [wr-cold | w32 | idx(as f32 cols)]
IDX_F32_COLS = BPC * 4 * 16 // 2          # int16 pairs packed in f32 words
COLD_COLS = (WR_COLS - HOT_COLS) + W32_COLS + IDX_F32_COLS
CW_W32 = WR_COLS - HOT_COLS               # w32 offset within cold pack
CW_IDX = CW_W32 + W32_COLS                # idx offset within cold pack
B_OFS = {name: i for i, (name, _p) in enumerate(B_SPEC)}
B_COLS = len(B_SPEC)
_WR_KM = {e[0]: (e[1], e[2]) for e in WR_SPEC}
_WR_DUP = {e[0]: e[3] for e in WR_SPEC}
_W32_KM = {e[0]: (e[1], e[2]) for e in W32_SPEC}


def _bd(w):
    """[k, m] -> [2k, 2m] block diagonal."""
    k, m = w.shape
    out = np.zeros((2 * k, 2 * m), np.float32)
    out[:k, :m] = w
    out[k:, m:] = w
    return out


def _host_packs(params):
    def _mlp_np(mlp):
        return [(np.asarray(w, np.float32), np.asarray(b, np.float32))
                for (w, b) in mlp]

    pin = [_mlp_np(m) for m in params["in"]]
    pmsg = [_mlp_np(m) for m in params["msg"]]
    pagg, pout = _mlp_np(params["agg"]), _mlp_np(params["out"])
    pf, pg = _mlp_np(params["f"]), _mlp_np(params["g"])

    wr = np.zeros((128, WR_COLS), np.float32)
    w32 = np.zeros((128, W32_COLS), np.float32)
    bp = np.zeros((128, B_COLS), np.float32)

    def put_r(name, w):
        k, m = _WR_KM[name]
        assert w.shape == (k, m), (name, w.shape, (k, m))
        wr[:k, WR_OFS[name]:WR_OFS[name] + m] = w
        if _WR_DUP[name]:
            assert k <= 64
            wr[64:64 + k, WR_OFS[name]:WR_OFS[name] + m] = w

    def put_32(name, w):
        k, m = _W32_KM[name]
        assert w.shape == (k, m), (name, w.shape)
        w32[:k, W32_OFS[name]:W32_OFS[name] + m] = w

    def put_b(name, v):
        bp[:v.shape[0], B_OFS[name]] = v

    for t in range(T):
        for l in range(3):
            put_r(f"in{t}_bd{l}", _bd(pin[t][l][0]))
            put_b(f"bin{t}_{l}", np.concatenate([pin[t][l][1]] * 2))
    put_r("msg1cat", np.concatenate([pmsg[0][0][0], pmsg[1][0][0]], axis=1))
    put_b("bmsg1cat", np.concatenate([pmsg[0][0][1], pmsg[1][0][1]]))
    m2bd = np.zeros((128, 128), np.float32)
    m2bd[:HID, :HID] = pmsg[0][1][0]
    m2bd[HID:, HID:] = pmsg[1][1][0]
    put_r("msg2bd", m2bd)
    put_b("bmsg2cat", np.concatenate([pmsg[0][1][1], pmsg[1][1][1]]))
    for t in range(T):
        put_r(f"msg3_{t}", pmsg[t][2][0])
    put_b("bmsg3cat", np.concatenate([pmsg[0][2][1], pmsg[1][2][1]]))
    msg3_bias_zero = all(not np.any(pmsg[t][2][1]) for t in range(T))
    front_zero = (all(not np.any(pin[t][l][1]) for t in range(T) for l in range(3))
                  and all(not np.any(pmsg[t][l][1]) for t in range(T) for l in range(2)))
    for l in range(3):
        put_r(f"agg_bd{l}", _bd(pagg[l][0]))
        put_b(f"bagg_{l}", np.concatenate([pagg[l][1]] * 2))
    for l in range(2):
        put_r(f"out_bd{l}", _bd(pout[l][0]))
        put_b(f"bout_{l}", np.concatenate([pout[l][1]] * 2))
    w3, b3 = pout[2]
    for og in range(4):
        rep = np.zeros((HID, 128), np.float32)
        brep = np.zeros(128, np.float32)
        for pp in range(128):
            o = og * 8 + pp // 16
            rep[:, pp] = w3[:, o]
            brep[pp] = b3[o]
        put_r(f"out3rep_{og}", rep)
        put_b(f"bout3rep_{og}", brep)
    wf1, bf1 = pf[0]
    for og in range(4):
        rep = np.zeros((128, HID), np.float32)
        for pp in range(128):
            rep[pp, :] = wf1[og * 8 + pp // 16, :] / 16.0
        put_32(f"f1rep_{og}", rep)
    put_b("bf_0", bf1)
    put_r("fL2bd", _bd(pf[1][0]))
    put_b("bf_1", np.concatenate([pf[1][1]] * 2))
    put_r("fL3bd", _bd(pf[2][0]))
    put_b("bf_2", np.concatenate([pf[2][1]] * 2))
    for l in range(3):
        put_32(f"g_{l}", pg[l][0])
        put_b(f"bg_{l}", pg[l][1])
    w32[32:64, W32_OFS["g_0"]:W32_OFS["g_0"] + HID] = pg[0][0]  # base-32 copy
    return wr, w32, bp, msg3_bias_zero, front_zero


# ---------------------------------------------------------------- kernel build

def _build(msg3_bias_zero, front_bias_zero):
    nc = bacc.Bacc("TRN2", target_bir_lowering=False, debug=False)

    nf_d = nc.dram_tensor("nf_d", [2 * F, T * N], F32R, kind="ExternalInput").ap()
    adjT_d = nc.dram_tensor("adjT_d", [BPC, T, N, N], FP16, kind="ExternalInput").ap()
    hot_d = nc.dram_tensor("hot_d", [128, HOT_COLS + B_COLS], F32R,
                           kind="ExternalInput").ap()
    cold_d = nc.dram_tensor("cold_d", [128, COLD_COLS], F32R,
                            kind="ExternalInput").ap()
    idx_d = nc.dram_tensor("idx_d", [128, BPC, 2, 32], I16, kind="ExternalInput").ap()
    out_d = nc.dram_tensor("out_d", [BPC, 1], FP32, kind="ExternalOutput").ap()

    with tile.TileContext(nc) as tc, ExitStack() as ctx:
        const = ctx.enter_context(tc.tile_pool(name="const", bufs=1))
        acts = ctx.enter_context(tc.tile_pool(name="acts", bufs=2))
        once = ctx.enter_context(tc.tile_pool(name="once", bufs=1))
        psA = ctx.enter_context(tc.tile_pool(name="psA", bufs=2, space="PSUM"))
        psB = ctx.enter_context(tc.tile_pool(name="psB", bufs=3, space="PSUM"))
        psC = ctx.enter_context(tc.tile_pool(name="psC", bufs=1, space="PSUM"))

        # ---- constant loads: hot pack (in-weights + biases) and node feats
        # first so compute starts immediately; cold pack next; the 8MB
        # adjacency streams behind in four 2MB strided DMAs.
        hot_s = const.tile([128, HOT_COLS + B_COLS], F32R)
        nc.sync.dma_start(hot_s[:], hot_d)
        nf_s = const.tile([2 * F, T, N], F32R)
        nc.sync.dma_start(nf_s[:, :, :],
                          nf_d.rearrange("f (t n) -> f t n", t=T))
        cold_s = const.tile([128, COLD_COLS], F32R)
        nc.sync.dma_start(cold_s[:], cold_d)
        adj_sb = const.tile([128, BPC, T, KC, N], FP16)
        for bl in range(BPC):
            for t in range(T):
                src = adjT_d[bl, t].rearrange("(kc p) n -> p kc n", p=128)
                nc.sync.dma_start(adj_sb[:, bl, t, :, :], src)

        def wslc(name, blk=0):
            k, m = _WR_KM[name]
            base = 64 * blk
            if blk:
                assert _WR_DUP[name] and k <= 64
            c = WR_OFS[name]
            if c < HOT_COLS:
                return hot_s[base:base + k, c:c + m]
            return cold_s[base:base + k, c - HOT_COLS:c - HOT_COLS + m]

        def w32slc(name):
            k, m = _W32_KM[name]
            c = CW_W32 + W32_OFS[name]
            return cold_s[:k, c:c + m].bitcast(FP32)

        def bslc(name, p=128):
            c = HOT_COLS + B_OFS[name]
            return hot_s[:p, c:c + 1].bitcast(FP32)

        idx_s = const.tile([128, BPC, 2, 32], I16)
        nc.sync.dma_start(idx_s[:], idx_d)

        from concourse.masks import make_identity
        ident = const.tile([128, 128], FP32)
        make_identity(nc, ident[:])
        # PE warm-up during the initial DMA window: ~3.5us of junk matmuls
        # releases the HAM clock throttle before the first real layer.
        for _w in range(8):
            jp = psB.tile([128, 128], FP32, tag="B")
            nc.tensor.matmul(jp[:], ident[:], ident[:], start=True, stop=True)
        wh_s = const.tile([128, H], FP16)   # msg-L3 fp16 weights, t at base 64*t
        for t in range(T):
            nc.scalar.copy(wh_s[64 * t:64 * (t + 1), :], wslc(f"msg3_{t}"))

        def bd_layer(wname, bname, rhs, kk, act_tag, out_dt=F32R,
                     act_func=LR, m=128, eng="act"):
            """One block-diag MLP layer on folded activations [128, N]."""
            ps = psA.tile([m, N], FP32, tag="A")
            for hlf in range(2):
                nc.tensor.matmul(ps[:, hlf * NH:(hlf + 1) * NH],
                                 wslc(wname), rhs[:kk, hlf * NH:(hlf + 1) * NH],
                                 start=True, stop=True)
            o = acts.tile([m, N], out_dt, tag=act_tag)
            if eng == "act":
                nc.scalar.activation(o[:], ps[:], act_func, bias=bslc(bname, m),
                                     scale=1.0, alpha=ALPHA)
            else:
                # zero-bias leaky relu on DVE: max(x, 0.01x), two ops
                tmp = acts.tile([m, N], FP32, tag="dvetmp")
                nc.vector.tensor_scalar_mul(tmp[:], ps[:], ALPHA)
                nc.vector.tensor_tensor(o[:], ps[:], tmp[:],
                                        op=mybir.AluOpType.max)
            return o

        # ---- input MLPs, summed over type -> h [128, N] f32r
        m_t = []
        for t in range(T):
            eng = "dve" if (USE_DVE_LRELU and t == 1 and front_bias_zero) else "act"
            a1 = bd_layer(f"in{t}_bd0", f"bin{t}_0", nf_s[:, t, :], 2 * F,
                          "tmp1" if t == 0 else "tmp1b", eng=eng)
            a2 = bd_layer(f"in{t}_bd1", f"bin{t}_1", a1, 128,
                          "tmp2" if t == 0 else "tmp2b", eng=eng)
            mt = bd_layer(f"in{t}_bd2", f"bin{t}_2", a2, 128, f"m{t}", eng=eng)
            m_t.append(mt)
        h = acts.tile([128, N], F32R, tag="h")
        for hlf in range(2):
            sl = slice(hlf * NH, (hlf + 1) * NH)
            nc.vector.tensor_add(h[:, sl], m_t[0][:, sl], m_t[1][:, sl])

        # ---- message-passing steps
        for _step in range(NSTEPS):
            a1b, a2b = [], []
            for bl in range(BPC):
                ps = psA.tile([128, N], FP32, tag="A")
                for hlf in range(2):
                    nc.tensor.matmul(
                        ps[:, hlf * NH:(hlf + 1) * NH], wslc("msg1cat", bl),
                        h[H * bl:H * (bl + 1), hlf * NH:(hlf + 1) * NH],
                        start=True, stop=True)
                a1 = acts.tile([128, N], F32R, tag="tmp1")
                if USE_DVE_LRELU and bl == 1 and front_bias_zero:
                    tmp = acts.tile([128, N], FP32, tag="dvetmp")
                    nc.vector.tensor_scalar_mul(tmp[:], ps[:], ALPHA)
                    nc.vector.tensor_tensor(a1[:], ps[:], tmp[:],
                                            op=mybir.AluOpType.max)
                else:
                    nc.scalar.activation(a1[:], ps[:], LR, bias=bslc("bmsg1cat"),
                                         scale=1.0, alpha=ALPHA)
                a1b.append(a1)
            for bl in range(BPC):
                ps = psA.tile([128, N], FP32, tag="A")
                for hlf in range(2):
                    nc.tensor.matmul(
                        ps[:, hlf * NH:(hlf + 1) * NH], wslc("msg2bd"),
                        a1b[bl][:, hlf * NH:(hlf + 1) * NH],
                        start=True, stop=True)
                a2 = acts.tile([128, N], FP16, tag="tmp2f16")
                if USE_DVE_LRELU and bl == 1 and front_bias_zero:
                    tmp = acts.tile([128, N], FP32, tag="dvetmp")
                    nc.vector.tensor_scalar_mul(tmp[:], ps[:], ALPHA)
                    nc.vector.tensor_tensor(a2[:], ps[:], tmp[:],
                                            op=mybir.AluOpType.max)
                else:
                    nc.scalar.activation(a2[:], ps[:], LR, bias=bslc("bmsg2cat"),
                                         scale=1.0, alpha=ALPHA)
                a2b.append(a2)
            # msg L3 -> node-major msg_B [128, bl, t, kc-block, H] fp16
            msgB = acts.tile([128, BPC, T, KC, H], FP16, tag="msgB")
            if msg3_bias_zero:
                pass
            else:
                # generic path: feature-major L3 + fp16 DMA transpose
                for bl in range(BPC):
                    ps = psA.tile([128, N], FP32, tag="A")
                    for t in range(T):
                        for hlf in range(2):
                            nc.tensor.matmul(
                                ps[HID * t:HID * (t + 1), hlf * NH:(hlf + 1) * NH],
                                wh_s[64 * t:64 * (t + 1), :],
                                a2b[bl][HID * t:HID * (t + 1),
                                        hlf * NH:(hlf + 1) * NH],
                                start=True, stop=True)
                    mf = acts.tile([128, N], FP16, tag="tmp1")
                    nc.scalar.activation(mf[:], ps[:], LR, bias=bslc("bmsg3cat"),
                                         scale=1.0, alpha=ALPHA)
                    for t in range(T):
                        for kc in range(KC):
                            nc.sync.dma_start(
                                msgB[:, bl, t, kc, :],
                                mf[H * t:H * (t + 1), kc * 128:(kc + 1) * 128],
                                transpose=True)
            # adjacency contraction, node-major (full 128-row PE + FWL):
            #   psN[n-block, h] += adjT-chunk.T @ msgB-chunk, then PE-transpose
            #   back to feature-major msgs[h + 64bl, n].
            msgs = acts.tile([128, N], F32R, tag="msgs")
            for bl in range(BPC):
                if msg3_bias_zero:
                    for t in range(T):
                        ps = psB.tile([128, KC, H], FP32, tag="B")
                        for kc in range(KC):
                            nc.tensor.matmul(
                                ps[:, kc, :],
                                a2b[bl][HID * t:HID * (t + 1),
                                        kc * 128:(kc + 1) * 128],
                                wh_s[64 * t:64 * (t + 1), :],
                                start=True, stop=True)
                        nc.scalar.activation(msgB[:, bl, t, :, :], ps[:], LR,
                                             bias=0.0, scale=1.0, alpha=ALPHA)
                psN = psB.tile([128, KC, H], FP32, tag="B")
                for nb in range(KC):
                    first = True
                    for t in range(T):
                        for kc in range(KC):
                            nc.tensor.matmul(
                                psN[:, nb, :],
                                adj_sb[:, bl, t, kc, nb * 128:(nb + 1) * 128],
                                msgB[:, bl, t, kc, :],
                                start=first, stop=(t == T - 1 and kc == KC - 1))
                            first = False
                msgsN = acts.tile([128, KC, H], FP32, tag="msgsN")
                nc.vector.tensor_copy(msgsN[:], psN[:])
                psT = psA.tile([64, N], FP32, tag="A")
                for nb in range(KC):
                    nc.tensor.transpose(psT[:, nb * 128:(nb + 1) * 128],
                                        msgsN[:, nb, :], ident[:])
                nc.scalar.copy(msgs[H * bl:H * (bl + 1), :], psT[:])

            a1 = bd_layer("agg_bd0", "bagg_0", msgs, 128, "tmp1")
            a2 = bd_layer("agg_bd1", "bagg_1", a1, 128, "tmp2")
            ps3 = psA.tile([128, N], FP32, tag="A")
            for hlf in range(2):
                nc.tensor.matmul(ps3[:, hlf * NH:(hlf + 1) * NH],
                                 wslc("agg_bd2"),
                                 a2[:, hlf * NH:(hlf + 1) * NH],
                                 start=True, stop=True)
            a3 = acts.tile([128, N], F32R, tag="tmp3")
            hn = acts.tile([128, N], F32R, tag="h")
            for hlf in range(2):
                sl = slice(hlf * NH, (hlf + 1) * NH)
                nc.scalar.activation(a3[:, sl], ps3[:, sl], LR,
                                     bias=bslc("bagg_2"), scale=1.0, alpha=ALPHA)
                nc.vector.tensor_add(hn[:, sl], h[:, sl], a3[:, sl])
            h = hn

        # ---- out MLP; L3 is partition-replicated for the gather
        a1 = bd_layer("out_bd0", "bout_0", h, 128, "tmp1")
        a2 = bd_layer("out_bd1", "bout_1", a1, 128, "tmp2")

        sw_big = once.tile([128, BPC, 2, 2 * S], FP32, tag="swb")
        for bl in range(BPC):
            for q in range(2):
                gdata = acts.tile([128, 2, N], FP32, tag="gdata")
                for half in range(2):
                    og = 2 * q + half
                    ps = psA.tile([128, N], FP32, tag="A")
                    for hlf in range(2):
                        nc.tensor.matmul(
                            ps[:, hlf * NH:(hlf + 1) * NH],
                            wslc(f"out3rep_{og}", bl),
                            a2[HID * bl:HID * (bl + 1), hlf * NH:(hlf + 1) * NH],
                            start=True, stop=True)
                    nc.scalar.activation(gdata[:, half, :], ps[:], LR,
                                         bias=bslc(f"bout3rep_{og}"),
                                         scale=1.0, alpha=ALPHA)
                nc.gpsimd.ap_gather(sw_big[:, bl, q, :], gdata[:],
                                    idx_s[:, bl, q, :], channels=128,
                                    num_elems=2 * N, d=1, num_idxs=2 * S)

        # ---- f MLP; L1 accumulates the 16x-replicated weight trick
        f1 = once.tile([128, S], F32R, tag="f1")
        for bl in range(BPC):
            psf = psB.tile([HID, S], FP32, tag="B")
            for og in range(4):
                nc.tensor.matmul(psf[:], w32slc(f"f1rep_{og}"),
                                 sw_big[:, bl, og // 2,
                                        (og % 2) * S:(og % 2 + 1) * S],
                                 start=(og == 0), stop=(og == 3))
            nc.scalar.activation(f1[HID * bl:HID * (bl + 1), :], psf[:], LR,
                                 bias=bslc("bf_0", HID), scale=1.0, alpha=ALPHA)
        # f-L2/L3 per bl so bl0's whole f/g tail hides under bl1's gathers
        c2 = WR_OFS["fL2bd"] - HOT_COLS
        c3 = WR_OFS["fL3bd"] - HOT_COLS
        pooled = once.tile([2 * OUT, 1], FP32, tag="pooled")
        f3 = once.tile([2 * OUT, S], FP32, tag="f3")
        for bl in range(BPC):
            base = HID * bl
            f2w = cold_s[base:base + HID, c2 + base:c2 + base + HID]
            psf2 = psB.tile([HID, S], FP32, tag="B")
            nc.tensor.matmul(psf2[:], f2w, f1[base:base + HID, :],
                             start=True, stop=True)
            f2 = once.tile([HID, S], F32R, tag=f"f2{bl}")
            nc.scalar.activation(f2[:], psf2[:], LR, bias=bslc("bf_1", HID),
                                 scale=1.0, alpha=ALPHA)
            f3w = cold_s[0:HID, c3:c3 + OUT]
            psf3 = psB.tile([OUT, S], FP32, tag="B")
            nc.tensor.matmul(psf3[:], f3w, f2[:], start=True, stop=True)
            nc.scalar.activation(f3[OUT * bl:OUT * (bl + 1), :], psf3[:], LR,
                                 bias=bslc("bf_2", OUT), scale=1.0, alpha=ALPHA,
                                 accum_out=pooled[OUT * bl:OUT * (bl + 1), :])

        # ---- g MLP: one chain per bl straight off the folded pooled vector
        # (avoids the SBUF->SBUF unfold DMAs on the critical tail)
        res = once.tile([1, BPC], FP32, tag="res")
        c0, m0 = W32_OFS["g_0"], HID
        for bl in range(BPC):
            base = OUT * bl
            g0w = cold_s[base:base + OUT,
                         CW_W32 + c0:CW_W32 + c0 + m0].bitcast(FP32)
            psg = psB.tile([HID, 1], FP32, tag="B")
            nc.tensor.matmul(psg[:], g0w,
                             pooled[base:base + OUT, :], start=True, stop=True)
            g1 = once.tile([HID, 1], FP32, tag=f"g1{bl}")
            nc.scalar.activation(g1[:], psg[:], LR, bias=bslc("bg_0", HID),
                                 scale=1.0, alpha=ALPHA)
            psg = psB.tile([HID, 1], FP32, tag="B")
            nc.tensor.matmul(psg[:], w32slc("g_1"), g1[:], start=True, stop=True)
            g2 = once.tile([HID, 1], FP32, tag=f"g2{bl}")
            nc.scalar.activation(g2[:], psg[:], LR, bias=bslc("bg_1", HID),
                                 scale=1.0, alpha=ALPHA)
            psg = psB.tile([1, 1], FP32, tag="B")
            nc.tensor.matmul(psg[:], w32slc("g_2"), g2[:], start=True, stop=True)
            nc.scalar.activation(res[:, bl:bl + 1], psg[:], IDENT,
                                 bias=bslc("bg_2", 1), scale=1.0, alpha=0.0)
        nc.sync.dma_start(out_d, res[:])

    nc.compile()
    return nc


_CACHE = {}


def _get_nc(msg3_bias_zero, front_bias_zero):
    key = (msg3_bias_zero, front_bias_zero)
    if key not in _CACHE:
        _CACHE[key] = _build(msg3_bias_zero, front_bias_zero)
    return _CACHE[key]


def _prep_inputs(node_feats, adj_mats, switch_idx, params):
    wr, w32, bp, msg3_bias_zero, front_bias_zero = _host_packs(params)
    hot = np.ascontiguousarray(
        np.concatenate([wr[:, :HOT_COLS], bp], axis=1))
    cold = np.zeros((128, COLD_COLS), np.float32)
    cold[:, :CW_W32] = wr[:, HOT_COLS:]
    cold[:, CW_W32:CW_W32 + W32_COLS] = w32
    # host-side layout prep (sharding): transpose features/adjacency, wrap idx
    nfT = np.ascontiguousarray(node_feats.transpose(0, 1, 3, 2))       # [B,T,F,N]
    adjT = np.ascontiguousarray(
        adj_mats.transpose(0, 1, 3, 2)).astype(np.float16)             # [B,T,N,N]
    idx16 = switch_idx.astype(np.int16)                                # [B,S,OUT]
    pr = np.arange(128)
    kk = np.arange(16)
    s_of = kk[None, :] * 16 + (pr % 16)[:, None]                       # [128,16]
    o_of = pr // 16                                                    # [128]
    # og-pair layout: per core row p, list i = j*16 + p%16 over j in [0,32);
    # first 256 entries gather og=2q (cols 0..N), next 256 og=2q+1 (cols N..2N)
    idx_w = np.zeros((B, 128, 2, 32), np.int16)
    for q in range(2):
        a = idx16[:, s_of, (16 * q + o_of)[:, None]]          # og=2q  [B,128,16]
        bseg = idx16[:, s_of, (16 * q + 8 + o_of)[:, None]] + N
        idx_w[:, :, q, :16] = a
        idx_w[:, :, q, 16:] = bseg

    in_maps = []
    for c in range(NCORES):
        bs = slice(c * BPC, (c + 1) * BPC)
        coldc = cold.copy()
        iw = np.ascontiguousarray(
            idx_w[bs].transpose(1, 0, 2, 3))                 # [128, BPC, 4, 16]
        coldc[:, CW_IDX:CW_IDX + IDX_F32_COLS] = (
            iw.reshape(128, -1).view(np.float32))
        # nf rows: [bl0 features | bl1 features], columns (t, n) flattened
        nfc = np.ascontiguousarray(
            nfT[bs].transpose(0, 2, 1, 3).reshape(2 * F, T * N))
        in_maps.append({
            "nf_d": nfc,
            "adjT_d": adjT[bs],
            "hot_d": hot, "cold_d": coldc,
            "idx_d": iw,
        })
    return in_maps, (msg3_bias_zero, front_bias_zero)


def kernel(node_feats, adj_mats, switch_idx, params):
    node_feats = np.asarray(node_feats, np.float32)
    adj_mats = np.asarray(adj_mats, np.float32)
    switch_idx = np.asarray(switch_idx)

    in_maps, flags = _prep_inputs(node_feats, adj_mats, switch_idx, params)
    nc = _get_nc(*flags)
    res = run_bass_kernel_spmd(nc, in_maps, core_ids=list(range(NCORES)))
    return np.concatenate([r["out_d"] for r in res.results], axis=0)
